# revision 17
# baseline (speedup 1.0000x reference)
"""Bahdanau attention on 8 Trainium2 cores (Bass/Tile), data-parallel over B.

reference (per batch b, all shapes full):
    hp  = hidden[0] @ W_h.T + b_h                    # (B, H)
    ep  = einsum('tbh,gh->btg', enc, W_e) + b_e      # (B, T, H)
    en  = tanh(hp[:, None, :] + ep)                  # (B, T, H)
    sc  = en @ v                                     # (B, T)
    out = softmax(sc, -1)[:, None, :]                # (B, 1, T)

Sharding: B=32 split 4-per-core across 8 cores; W_h/W_e/b/v replicated.
Per-core kernel layout: tokens of one batch are processed in groups of 512;
enc tiles are PE-transposed to put H on partitions; ep accumulates over
8 h-chunks in PSUM as [g=128, tok=512] via fp32r matmuls (full PE rate,
~tf32 accuracy); ACT applies tanh with the per-partition bias
hp^T[:, b] + b_h + b_e; a [128x4] fp32r matmul against v4 (v in column b,
zeros elsewhere) reduces over g so batch b's scores land on PSUM partition
b; rows are assembled into a [4, 2048] score tile by SBUF-to-SBUF DMA and
softmax runs there on 4 partitions.
"""

import sys
from contextlib import ExitStack

import numpy as np

try:
    import concourse  # noqa: F401
except ImportError:  # pragma: no cover
    sys.path.insert(0, "/opt/trn_rl_repo")

import concourse.tile as tile
from concourse import bacc, mybir
from concourse.bass import ts
from concourse.bass_utils import run_bass_kernel_spmd
from concourse.masks import make_identity

H = 1024
T = 2048
B = 32
NCORES = 8
BC = B // NCORES          # batches per core
HC = H // 128             # h chunks
GC = H // 128             # g chunks
TOK = 512                 # tokens per group (one batch each)
SUB = TOK // 128          # 128-token subtiles per group
NGRP_PER_B = T // TOK
NGRP = BC * NGRP_PER_B

F32 = mybir.dt.float32
F32R = mybir.dt.float32r
AF = mybir.ActivationFunctionType
AX = mybir.AxisListType


def build_kernel_nc(reps=1):
    nc = bacc.Bacc(
        "TRN2",
        target_bir_lowering=False,
        debug=False,
        enable_asserts=False,
        num_devices=NCORES,
    )
    enc = nc.dram_tensor("enc", [T, BC, H], F32, kind="ExternalInput").ap()
    hid = nc.dram_tensor("hid", [BC, H], F32, kind="ExternalInput").ap()
    w_e = nc.dram_tensor("W_e", [H, H], F32, kind="ExternalInput").ap()
    w_h = nc.dram_tensor("W_h", [H, H], F32, kind="ExternalInput").ap()
    b_h = nc.dram_tensor("b_h", [H], F32, kind="ExternalInput").ap()
    b_e = nc.dram_tensor("b_e", [H], F32, kind="ExternalInput").ap()
    v = nc.dram_tensor("v", [H], F32, kind="ExternalInput").ap()
    out = nc.dram_tensor("out", [BC, T], F32, kind="ExternalOutput").ap()

    with tile.TileContext(nc) as tc:
        _kernel_body(tc, enc, hid, w_e, w_h, b_h, b_e, v, out, reps=reps)
    nc.compile()
    return nc


def _kernel_body(tc, enc, hid, w_e, w_h, b_h, b_e, v, out, reps=1):
    nc = tc.nc
    with ExitStack() as ctx:
        singles = ctx.enter_context(tc.tile_pool(name="singles", bufs=1))
        enc_pool = ctx.enter_context(tc.tile_pool(name="enc_nat", bufs=2 * SUB))
        encT_pool = ctx.enter_context(tc.tile_pool(name="encT", bufs=2))
        energy_pool = ctx.enter_context(tc.tile_pool(name="energy", bufs=3))
        scrow_pool = ctx.enter_context(tc.tile_pool(name="scrow", bufs=2))
        trps_pool = ctx.enter_context(
            tc.tile_pool(name="trps", bufs=3, space="PSUM")
        )
        ep_pool = ctx.enter_context(tc.tile_pool(name="epps", bufs=3, space="PSUM"))
        sc_pool = ctx.enter_context(tc.tile_pool(name="scps", bufs=2, space="PSUM"))

        identity = singles.tile([128, 128], F32)
        make_identity(nc, identity[:])

        # ---- persistent SBUF tensors -------------------------------------
        WeT = singles.tile([128, HC, H], F32R)     # WeT[h, hc, g] = W_e[g, 128*hc+h]
        WhT = singles.tile([128, HC, H], F32)
        hidT = singles.tile([128, HC, BC], F32)    # hidT[h, hc, b] = hid[b, 128*hc+h]
        bias_all = singles.tile([128, GC, BC], F32)  # hp^T + b_h + b_e
        v_sb = singles.tile([128, GC], F32)        # v[gc*128+p] at [p, gc]
        # v4[:, gc, b, :] is a [128, BC] stationary operand whose column b
        # holds the v chunk and the rest are zero -> batch b's scores land
        # on PSUM partition b (fp32r matmuls require dst partition 0).
        v4f = singles.tile([128, GC, BC, BC], F32)
        v4 = singles.tile([128, GC, BC, BC], F32R)
        bsum = singles.tile([128, GC], F32)        # (b_h + b_e) chunked
        scores = singles.tile([BC, T], F32)        # row b = batch b scores
        probs = singles.tile([BC, T], F32)
        negmax = singles.tile([BC, 1], F32)
        sums = singles.tile([BC, 1], F32)
        rsum = singles.tile([BC, 1], F32)

        # ---- stage 0: weights transpose + hp + biases --------------------
        bh_sb = singles.tile([128, GC], F32)
        be_sb = singles.tile([128, GC], F32)
        nc.sync.dma_start(out=bh_sb[:], in_=b_h.rearrange("(c p) -> p c", p=128))
        nc.sync.dma_start(out=be_sb[:], in_=b_e.rearrange("(c p) -> p c", p=128))
        nc.sync.dma_start(out=v_sb[:], in_=v.rearrange("(c p) -> p c", p=128))
        nc.vector.tensor_add(bsum[:], bh_sb[:], be_sb[:])
        nc.gpsimd.memset(v4f[:], 0.0)
        for b in range(BC):
            for gc in range(GC):
                nc.vector.tensor_copy(v4f[:, gc, b, b : b + 1], v_sb[:, gc : gc + 1])
        nc.vector.tensor_copy(v4[:], v4f[:])

        with tc.tile_pool(name="stage0", bufs=4) as wload:
            for w_src, w_dst in ((w_e, WeT), (w_h, WhT)):
                for gc in range(GC):
                    wn = wload.tile([128, H], F32, tag="wn")
                    nc.sync.dma_start(out=wn[:], in_=w_src[ts(gc, 128), :])
                    for hc in range(HC):
                        tp = trps_pool.tile([128, 128], F32, tag="tr")
                        nc.tensor.transpose(tp[:], wn[:, ts(hc, 128)], identity[:])
                        nc.vector.tensor_copy(w_dst[:, hc, ts(gc, 128)], tp[:])

            hid_nat = wload.tile([BC, H], F32, tag="hid")
            nc.sync.dma_start(out=hid_nat[:], in_=hid[:, :])
            for hc in range(HC):
                tph = trps_pool.tile([128, BC], F32, tag="tr")
                nc.tensor.transpose(
                    tph[:], hid_nat[:, ts(hc, 128)], identity[0:BC, 0:BC]
                )
                nc.vector.tensor_copy(hidT[:, hc, :], tph[:])

            # hp^T[g, b] accumulated over h chunks (fp32, tiny N)
            for gc in range(GC):
                hp_ps = trps_pool.tile([128, BC], F32, tag="tr")
                for hc in range(HC):
                    nc.tensor.matmul(
                        hp_ps[:],
                        WhT[:, hc, ts(gc, 128)],
                        hidT[:, hc, :],
                        start=(hc == 0),
                        stop=(hc == HC - 1),
                    )
                nc.vector.tensor_scalar(
                    out=bias_all[:, gc, :],
                    in0=hp_ps[:],
                    scalar1=bsum[:, gc : gc + 1],
                    scalar2=None,
                    op0=mybir.AluOpType.add,
                )

        # ---- main loop: 16 groups of 512 tokens --------------------------
        # Software-pipelined so the in-order PE queue never waits on ACT:
        #   iteration g emits: DMA(g+2), transposes(g+1), ep/sc chain(g)
        # with sc(gc-1) emitted after ep(gc) so tanh(gc-1) is long done.
        n_total = reps * NGRP

        def issue_load(grp):
            g = grp % NGRP
            b = g // NGRP_PER_B
            t0 = (g % NGRP_PER_B) * TOK
            en_nat = []
            for s in range(SUB):
                en = enc_pool.tile([128, H], F32, tag="en")
                nc.sync.dma_start(
                    out=en[:], in_=enc[t0 + s * 128 : t0 + (s + 1) * 128, b, :]
                )
                en_nat.append(en)
            return en_nat

        def issue_transposes(en_nat):
            encT = encT_pool.tile([128, HC, TOK], F32R)
            for hc in range(HC):
                tp = trps_pool.tile([128, TOK], F32, tag="tr")
                for s in range(SUB):
                    nc.tensor.transpose(
                        tp[:, ts(s, 128)], en_nat[s][:, ts(hc, 128)], identity[:]
                    )
                nc.vector.tensor_copy(encT[:, hc, :], tp[:])
            return encT

        loads = [issue_load(0), issue_load(1)]
        encT_cur = issue_transposes(loads[0])
        carry = None  # deferred final sc-mm of the previous group

        def flush_carry(c):
            c_sc_ps, c_gc, c_energy, c_b, c_t0 = c
            nc.tensor.matmul(
                c_sc_ps[:], v4[:, c_gc, c_b, :], c_energy[:],
                start=False, stop=True,
            )
            sc_sb = scrow_pool.tile([BC, TOK], F32)
            nc.vector.tensor_copy(sc_sb[:], c_sc_ps[:])
            nc.sync.dma_start(
                out=scores[c_b : c_b + 1, c_t0 : c_t0 + TOK],
                in_=sc_sb[c_b : c_b + 1, :],
            )

        for grp in range(n_total):
            g = grp % NGRP
            b = g // NGRP_PER_B
            t0 = (g % NGRP_PER_B) * TOK

            if grp + 2 < n_total:
                loads.append(issue_load(grp + 2))
            encT_next = None
            if grp + 1 < n_total:
                encT_next = issue_transposes(loads[grp + 1])
            if carry is not None:
                flush_carry(carry)
                carry = None

            sc_ps = sc_pool.tile([BC, TOK], F32)
            pending = None
            for gc in range(GC):
                ep_ps = ep_pool.tile([128, TOK], F32)
                for hc in range(HC):
                    nc.tensor.matmul(
                        ep_ps[:],
                        WeT[:, hc, ts(gc, 128)],
                        encT_cur[:, hc, :],
                        start=(hc == 0),
                        stop=(hc == HC - 1),
                    )
                if pending is not None:
                    pc, penergy = pending
                    nc.tensor.matmul(
                        sc_ps[:], v4[:, pc, b, :], penergy[:],
                        start=(pc == 0), stop=False,
                    )
                energy = energy_pool.tile([128, TOK], F32R)
                nc.scalar.activation(
                    out=energy[:],
                    in_=ep_ps[:],
                    func=AF.Tanh,
                    bias=bias_all[:, gc, b : b + 1],
                    scale=1.0,
                )
                pending = (gc, energy)
            pc, penergy = pending
            carry = (sc_ps, pc, penergy, b, t0)
            encT_cur = encT_next

        flush_carry(carry)

        # ---- softmax over T (rows 32*b are live) -------------------------
        nc.vector.tensor_reduce(
            out=negmax[:], in_=scores[:], axis=AX.X, op=mybir.AluOpType.max,
            negate=True,
        )
        nc.scalar.activation(
            out=probs[:], in_=scores[:], func=AF.Exp, bias=negmax[:], scale=1.0,
            accum_out=sums[:],
        )
        nc.vector.reciprocal(out=rsum[:], in_=sums[:])
        nc.vector.tensor_scalar_mul(probs[:], probs[:], rsum[:])

        nc.sync.dma_start(out=out[:, :], in_=probs[:])


_NC_CACHE = None


def _get_nc():
    global _NC_CACHE
    if _NC_CACHE is None:
        _NC_CACHE = build_kernel_nc()
    return _NC_CACHE


def make_in_maps(hidden, encoder_outputs, W_h, b_h, W_e, b_e, v):
    hidden = np.asarray(hidden, dtype=np.float32)
    enc = np.asarray(encoder_outputs, dtype=np.float32)
    W_h = np.ascontiguousarray(np.asarray(W_h, dtype=np.float32))
    W_e = np.ascontiguousarray(np.asarray(W_e, dtype=np.float32))
    b_h = np.ascontiguousarray(np.asarray(b_h, dtype=np.float32))
    b_e = np.ascontiguousarray(np.asarray(b_e, dtype=np.float32))
    v = np.ascontiguousarray(np.asarray(v, dtype=np.float32))
    hid0 = hidden.reshape(B, H)
    in_maps = []
    for c in range(NCORES):
        in_maps.append(
            {
                "enc": np.ascontiguousarray(enc[:, c * BC : (c + 1) * BC, :]),
                "hid": np.ascontiguousarray(hid0[c * BC : (c + 1) * BC, :]),
                "W_e": W_e,
                "W_h": W_h,
                "b_h": b_h,
                "b_e": b_e,
                "v": v,
            }
        )
    return in_maps


def kernel(hidden, encoder_outputs, W_h, b_h, W_e, b_e, v):
    nc = _get_nc()
    in_maps = make_in_maps(hidden, encoder_outputs, W_h, b_h, W_e, b_e, v)
    res = run_bass_kernel_spmd(nc, in_maps, list(range(NCORES)))
    full = np.concatenate([res.results[c]["out"] for c in range(NCORES)], axis=0)
    return full[:, None, :].astype(np.float32)
